# revision 10
# baseline (speedup 1.0000x reference)
"""Trainium2 Bass kernel for CUDALinearAttention (b=4, t=4096, d=1024, h=16).

Sharding: 8 NeuronCores = 4 batches x 2 head-groups (8 heads / 512 out-dims each).
Each core is fully independent (KV aggregation is per-head); no collectives.

Per-core pipeline (all matmuls bf16, fp32 PSUM accumulation):
  T: x [4096,1024] loaded token-major, transposed on-chip (PE identity matmul)
     to xT (d-on-partitions, bf16) -- both projection operands need K=d on
     partitions.
  A: k/v projections token-major: out[t,o] tiles; phi(k)=exp(min(k,0))+relu(k),
     masked; v masked, stored per-head 65-wide with a trailing mask/ones column.
  B: kv_h = kf_h^T @ [v_h | m] accumulated over t in PSUM; two heads share a
     bank via column-group packing; col 64 gives z_h = sum_t kf_h for free.
  C: q projection head-major (W stationary, xT moving): qfT[o,t] tiles, so the
     num/den matmuls need qf K(=hd)-major -- which this layout already is.
  D: den = qfT^T @ Z (block-sparse Z built from z_h) in one accumulated matmul
     per t-chunk; num_h = qfT_h^T @ kv_h (row-group packed, 2 heads concurrent);
     out = num * recip(max(den,1e-6)) * mask on DVE; DMA out.
"""

import sys

sys.path.insert(0, "/opt/trn_rl_repo")

import numpy as np
import ml_dtypes

import concourse.bass as bass
import concourse.tile as tile
from concourse import bacc, mybir
from concourse.bass_utils import run_bass_kernel_spmd
from concourse.masks import make_identity

F32 = mybir.dt.float32
BF16 = mybir.dt.bfloat16
AF = mybir.ActivationFunctionType
ALU = mybir.AluOpType

T = 4096
D = 1024
HG = 512  # per-core output dims (8 heads x 64)
KC = 8  # contraction chunks of 128 over D
TC = 32  # token chunks of 128
T512 = 8  # token chunks of 512
OC = 4  # output-dim chunks of 128 within HG (= head pairs)
HALVES = 2  # t mega-halves (keeps xT at 32KB/partition)
TCH = TC // HALVES
T5H = T512 // HALVES


def _build_program(has_bias: bool):
    import os

    stages = os.environ.get("LK_STAGES", "TABCD")
    nc = bacc.Bacc("TRN2", target_bir_lowering=False, debug=False)

    xb = nc.dram_tensor("xb", [T, D], F32, kind="ExternalInput")
    maskb = nc.dram_tensor("maskb", [T], F32, kind="ExternalInput")
    wqt = nc.dram_tensor("wqt", [D, HG], BF16, kind="ExternalInput")
    wkt = nc.dram_tensor("wkt", [D, HG], BF16, kind="ExternalInput")
    wvt = nc.dram_tensor("wvt", [D, HG], BF16, kind="ExternalInput")
    bqp = nc.dram_tensor("bqp", [HG], F32, kind="ExternalInput")
    bkr = nc.dram_tensor("bkr", [1, HG], BF16, kind="ExternalInput")
    bvr = nc.dram_tensor("bvr", [1, HG], BF16, kind="ExternalInput")
    outd = nc.dram_tensor("out", [T, HG], F32, kind="ExternalOutput")

    with tile.TileContext(nc) as tc:
        with (
            tc.tile_pool(name="const", bufs=1) as constp,
            tc.tile_pool(name="wp", bufs=1) as wp,
            tc.tile_pool(name="xTp", bufs=1) as xTp,
            tc.tile_pool(name="kfp", bufs=1) as kfp,
            tc.tile_pool(name="vap", bufs=1) as vap,
            tc.tile_pool(name="qfp", bufs=1) as qfp,
            tc.tile_pool(name="kvsp", bufs=1) as kvsp,
            tc.tile_pool(name="stage", bufs=2) as stage,
            tc.tile_pool(name="ptmp", bufs=2) as ptmp,
            tc.tile_pool(name="outp", bufs=2) as outp,
            tc.tile_pool(name="rdp", bufs=2) as rdp,
            tc.tile_pool(name="bigp", bufs=3, space="PSUM") as bigp,
            tc.tile_pool(name="kvp", bufs=1, space="PSUM") as kvpp,
            tc.tile_pool(name="smallp", bufs=2, space="PSUM") as smallp,
        ):
            # ---- constants ----
            ident = constp.tile([128, 128], F32)
            make_identity(nc, ident[:])
            mask_sb = constp.tile([128, TC], F32)
            nc.sync.dma_start(mask_sb[:], maskb.ap().rearrange("(a p) -> p a", p=128))
            bq_sb = constp.tile([128, OC], F32)
            nc.sync.dma_start(bq_sb[:], bqp.ap().rearrange("(a p) -> p a", p=128))
            if has_bias:
                ones_b = constp.tile([1, 128], BF16)
                nc.vector.memset(ones_b[:], 1.0)
                bk_sb = constp.tile([1, HG], BF16)
                nc.sync.dma_start(bk_sb[:], bkr.ap())
                bv_sb = constp.tile([1, HG], BF16)
                nc.sync.dma_start(bv_sb[:], bvr.ap())

            # ---- weights (host pre-transposed to [D, HG]) ----
            w_sb = {}
            for name, dram in (("q", wqt), ("k", wkt), ("v", wvt)):
                w = wp.tile([128, KC, HG], BF16, tag=f"w{name}")
                nc.sync.dma_start(w[:], dram.ap().rearrange("(kc p) n -> p kc n", p=128))
                w_sb[name] = w

            # ---- big persistent activations ----
            kf = kfp.tile([128, TC, HG], BF16)
            va = vap.tile([128, TC, 8 * 65], BF16)
            qf = qfp.tile([128, OC, T], BF16)
            # kvs[:, j, hh, :]: rows hh*64..hh*64+63 hold kv of head 2j+hh,
            # complementary rows stay zero so num matmuls can contract K=128
            # from base partition 0 (row-group-64 operands crash hardware).
            kvs = kvsp.tile([128, OC, 2, 64], BF16)
            zmat = kvsp.tile([128, OC, 8], BF16, tag="zmat")
            nc.vector.memset(kvs[:], 0.0)
            nc.vector.memset(zmat[:], 0.0)

            for half in range(HALVES):
                xT = xTp.tile([128, KC, T // HALVES], BF16, tag="xT")

                # ---- phase T: load + on-chip transpose of x ----
                for tl in range(TCH if "T" in stages else 0):
                    t_c = half * TCH + tl
                    xs = stage.tile([128, D], F32, tag="xs")
                    nc.sync.dma_start(xs[:], xb.ap()[t_c * 128 : (t_c + 1) * 128, :])
                    for grp in range(2):
                        tp = bigp.tile([128, 512], F32, tag="big")
                        for s in range(4):
                            kc = grp * 4 + s
                            nc.tensor.matmul(
                                tp[:, s * 128 : (s + 1) * 128],
                                xs[:, kc * 128 : (kc + 1) * 128],
                                ident[:],
                                is_transpose=True,
                                start=(s == 0),
                                stop=(s == 3),
                            )
                        dst = xT[:, grp * 4 : (grp + 1) * 4, tl * 128 : (tl + 1) * 128]
                        src = tp[:].rearrange("p (a b) -> p a b", b=128)
                        if grp == 0:
                            nc.vector.tensor_copy(dst, src)
                        else:
                            nc.scalar.copy(dst, src)

                # ---- phase A: k/v projections (token-major) + phi/mask ----
                for tl in range(TCH if "A" in stages else 0):
                    t_c = half * TCH + tl
                    m_col = mask_sb[:, t_c : t_c + 1]

                    kp = bigp.tile([128, 512], F32, tag="big")
                    for kc in range(KC):
                        nc.tensor.matmul(
                            kp[:],
                            xT[:, kc, tl * 128 : (tl + 1) * 128],
                            w_sb["k"][:, kc, :],
                            start=(kc == 0),
                            stop=(kc == KC - 1 and not has_bias),
                        )
                    if has_bias:
                        nc.tensor.matmul(kp[:], ones_b[:], bk_sb[:], start=False, stop=True)
                    kmin = ptmp.tile([128, 512], F32, tag="mn")
                    nc.vector.tensor_scalar_min(kmin[:], kp[:], 0.0)
                    ke = ptmp.tile([128, 512], F32, tag="ex")
                    nc.scalar.activation(ke[:], kmin[:], AF.Exp)
                    kr = ptmp.tile([128, 512], F32, tag="rl")
                    # relu(k * m) == m * relu(k) for m >= 0
                    nc.scalar.activation(kr[:], kp[:], AF.Relu, scale=m_col)
                    # kf = (ke * m) + (kr * m), both already/here masked
                    nc.vector.scalar_tensor_tensor(
                        kf[:, t_c, :], ke[:], m_col, kr[:], op0=ALU.mult, op1=ALU.add
                    )

                    vp = bigp.tile([128, 512], F32, tag="big")
                    for kc in range(KC):
                        nc.tensor.matmul(
                            vp[:],
                            xT[:, kc, tl * 128 : (tl + 1) * 128],
                            w_sb["v"][:, kc, :],
                            start=(kc == 0),
                            stop=(kc == KC - 1 and not has_bias),
                        )
                    if has_bias:
                        nc.tensor.matmul(vp[:], ones_b[:], bv_sb[:], start=False, stop=True)
                    va_t = va[:, t_c, :].rearrange("p (h c) -> p h c", c=65)
                    nc.scalar.mul(
                        va_t[:, :, 0:64],
                        vp[:].rearrange("p (h c) -> p h c", c=64),
                        m_col,
                    )
                    nc.vector.tensor_copy(
                        va_t[:, :, 64:65], m_col.broadcast_to((128, 8, 1))
                    )

                # ---- phase C: q projection (head-major) + phi ----
                for oc in range(OC if "C" in stages else 0):
                    for t5l in range(T5H):
                        t5 = half * T5H + t5l
                        qp = bigp.tile([128, 512], F32, tag="big")
                        for kc in range(KC):
                            nc.tensor.matmul(
                                qp[:],
                                w_sb["q"][:, kc, oc * 128 : (oc + 1) * 128],
                                xT[:, kc, t5l * 512 : (t5l + 1) * 512],
                                start=(kc == 0),
                                stop=(kc == KC - 1),
                            )
                        b_col = bq_sb[:, oc : oc + 1]
                        qmin = ptmp.tile([128, 512], F32, tag="mn")
                        if has_bias:
                            nc.vector.tensor_scalar(
                                qmin[:], qp[:], b_col, 0.0, op0=ALU.add, op1=ALU.min
                            )
                        else:
                            nc.vector.tensor_scalar_min(qmin[:], qp[:], 0.0)
                        qe = ptmp.tile([128, 512], F32, tag="ex")
                        nc.scalar.activation(qe[:], qmin[:], AF.Exp)
                        qr = ptmp.tile([128, 512], F32, tag="rl")
                        if has_bias:
                            nc.scalar.activation(qr[:], qp[:], AF.Relu, bias=b_col)
                        else:
                            nc.scalar.activation(qr[:], qp[:], AF.Relu)
                        nc.vector.tensor_add(
                            qf[:, oc, t5 * 512 : (t5 + 1) * 512], qe[:], qr[:]
                        )

            # ---- phase B: per-head-pair KV accumulation over all t ----
            for j in range(OC if "B" in stages else 0):
                kvp_t = kvpp.tile([128, 65], F32, tag="kvp")
                # sequential accumulation groups per column-half (a bank-wide
                # zero region only supports one open group at a time)
                for hh in range(2):
                    h = 2 * j + hh
                    for t_c in range(TC):
                        nc.tensor.matmul(
                            kvp_t[hh * 64 : (hh + 1) * 64, :],
                            kf[:, t_c, h * 64 : (h + 1) * 64],
                            va[:, t_c, h * 65 : (h + 1) * 65],
                            start=(t_c == 0),
                            stop=(t_c == TC - 1),
                        )
                for hh in range(2):
                    sl = slice(hh * 64, (hh + 1) * 64)
                    nc.vector.tensor_copy(kvs[sl, j, hh, :], kvp_t[sl, 0:64])
                    nc.vector.tensor_copy(zmat[sl, j, 2 * j + hh : 2 * j + hh + 1], kvp_t[sl, 64:65])


            # ---- phase D: num/den + normalize + store ----
            dflags = set(os.environ.get("LK_D", "").split(",")) - {""}
            for t_c in range(TC if "D" in stages else 0):
                m_col = mask_sb[:, t_c : t_c + 1]
                rden = rdp.tile([128, 8], F32, tag="rd")
                if "noden" in dflags:
                    nc.vector.memset(rden[:], 1.0)
                else:
                    den_t = smallp.tile([128, 8], F32, tag="den")
                    for oc in range(OC):
                        nc.tensor.matmul(
                            den_t[:],
                            qf[:, oc, t_c * 128 : (t_c + 1) * 128],
                            zmat[:, oc, :],
                            start=(oc == 0),
                            stop=(oc == OC - 1),
                        )
                    nc.vector.tensor_scalar_max(rden[:], den_t[:], 1e-6)
                    if "norecip" not in dflags:
                        nc.vector.reciprocal(rden[:], rden[:])

                ot = outp.tile([128, HG], F32, tag="ot")
                for j in range(OC):
                    nm = smallp.tile([128, 2, 64], F32, tag="num")
                    for hh in range(1 if "noodd" in dflags else 2):
                        nc.tensor.matmul(
                            nm[:, hh, :],
                            qf[:, j, t_c * 128 : (t_c + 1) * 128],
                            kvs[:, j, hh, :],
                            start=(hh == 0),
                            stop=(hh == 1 or "noodd" in dflags),
                        )
                    for hh in range(2):
                        h = 2 * j + hh
                        if "nomask" in dflags:
                            nc.vector.tensor_scalar_mul(
                                ot[:, h * 64 : (h + 1) * 64],
                                nm[:, hh, :],
                                rden[:, h : h + 1],
                            )
                        else:
                            nc.vector.tensor_scalar(
                                ot[:, h * 64 : (h + 1) * 64],
                                nm[:, hh, :],
                                rden[:, h : h + 1],
                                m_col,
                                op0=ALU.mult,
                                op1=ALU.mult,
                            )
                nc.sync.dma_start(outd.ap()[t_c * 128 : (t_c + 1) * 128, :], ot[:])

    nc.compile()
    return nc


_PROGRAM_CACHE = {}


def _get_program(has_bias: bool):
    if has_bias not in _PROGRAM_CACHE:
        _PROGRAM_CACHE[has_bias] = _build_program(has_bias)
    return _PROGRAM_CACHE[has_bias]


def _prep_inputs(x, mask, Wq, bq, Wk, bk, Wv, bv):
    """Slice + lay out per-core inputs. Core c -> batch c//2, head-group c%2."""
    bf16 = ml_dtypes.bfloat16
    in_maps = []
    for c in range(8):
        bi, hg = c // 2, c % 2
        sl = slice(hg * HG, (hg + 1) * HG)
        in_maps.append(
            {
                "xb": np.ascontiguousarray(x[bi]).astype(np.float32, copy=False),
                "maskb": np.ascontiguousarray(mask[bi]).astype(np.float32, copy=False),
                "wqt": np.ascontiguousarray(Wq[sl, :].T).astype(bf16),
                "wkt": np.ascontiguousarray(Wk[sl, :].T).astype(bf16),
                "wvt": np.ascontiguousarray(Wv[sl, :].T).astype(bf16),
                "bqp": np.ascontiguousarray(bq[sl]).astype(np.float32, copy=False),
                "bkr": np.ascontiguousarray(bk[sl]).astype(bf16).reshape(1, HG),
                "bvr": np.ascontiguousarray(bv[sl]).astype(bf16).reshape(1, HG),
            }
        )
    return in_maps


def kernel(x, mask, Wq, bq, Wk, bk, Wv, bv, n_heads, **run_kwargs):
    x = np.asarray(x)
    mask = np.asarray(mask)
    Wq, bq = np.asarray(Wq), np.asarray(bq)
    Wk, bk = np.asarray(Wk), np.asarray(bk)
    Wv, bv = np.asarray(Wv), np.asarray(bv)
    b, t, d = x.shape
    assert (b, t, d) == (4, T, D) and int(n_heads) == 16, (
        f"kernel hardcoded for (4,{T},{D}) h=16, got {(b, t, d)} h={n_heads}"
    )

    has_bias = bool(np.any(bq) or np.any(bk) or np.any(bv))
    nc = _get_program(has_bias)
    in_maps = _prep_inputs(x, mask, Wq, bq, Wk, bk, Wv, bv)
    res = run_bass_kernel_spmd(nc, in_maps, core_ids=list(range(8)), **run_kwargs)

    out = np.empty((4, T, D), dtype=np.float32)
    for c in range(8):
        bi, hg = c // 2, c % 2
        out[bi, :, hg * HG : (hg + 1) * HG] = res.results[c]["out"]
    if run_kwargs:
        kernel.last_results = res
    return out


# revision 11
# speedup vs baseline: 1.2055x; 1.2055x over previous
"""Trainium2 Bass kernel for CUDALinearAttention (b=4, t=4096, d=1024, h=16).

Sharding: 8 NeuronCores = 4 batches x 2 head-groups (8 heads / 512 out-dims each).
Each core is fully independent (KV aggregation is per-head); no collectives.

Per-core pipeline (all matmuls bf16, fp32 PSUM accumulation):
  T: x loaded token-major, cast to bf16 (DVE), transposed on-chip to xT
     (d-on-partitions) -- PE identity-matmul (8 blocks packed per PSUM bank)
     or DMA xbar transpose (LK_TMODE=dma).
  A: k/v projections token-major; phi(k)=exp(min(k,0))+relu(k), masked;
     v masked, stored per-pair as [v_h0 | m | v_h1 | m] (130-wide blocks).
  B: per head pair j one matmul chain over t: lhsT = kf pair cols [128,128],
     rhs = va pair block [128,130] -> kv of both heads in row-halves, z in
     col 64 (garbage halves never read). Evicted zero-padded into kvs so
     downstream matmuls contract K=128 from base partition 0 (row-group-64
     operands crash hardware).
  C: q projection head-major (W stationary, xT moving): qfT[o,t] -- already
     K(=hd)-major for num/den.
  D: one matmul per pair/chunk: rhs = kvs[:,j,:] = [kv_h0|kv_h1|z0|z1]
     [128,130] -> cols 0..127 = num both heads, 128/129 = den; then
     out = num * recip(max(den,1e-6)) * mask on DVE; DMA out.
"""

import os
import sys

sys.path.insert(0, "/opt/trn_rl_repo")

import numpy as np
import ml_dtypes

import concourse.bass as bass
import concourse.tile as tile
from concourse import bacc, mybir
from concourse.bass_utils import run_bass_kernel_spmd
from concourse.masks import make_identity

F32 = mybir.dt.float32
BF16 = mybir.dt.bfloat16
AF = mybir.ActivationFunctionType
ALU = mybir.AluOpType

T = 4096
D = 1024
HG = 512  # per-core output dims (8 heads x 64)
KC = 8  # contraction chunks of 128 over D
TC = 32  # token chunks of 128
OC = 4  # output-dim chunks of 128 within HG (= head pairs)
HALVES = 4  # t mega-chunks (xT quarter double-buffered)
TCH = TC // HALVES
T5H = (T // 512) // HALVES


def _build_program(has_bias: bool):
    stages = os.environ.get("LK_STAGES", "TABCD")
    tmode = os.environ.get("LK_TMODE", "pe")
    nc = bacc.Bacc("TRN2", target_bir_lowering=False, debug=False)

    xb = nc.dram_tensor("xb", [T, D], F32, kind="ExternalInput")
    maskb = nc.dram_tensor("maskb", [T], F32, kind="ExternalInput")
    wqt = nc.dram_tensor("wqt", [D, HG], BF16, kind="ExternalInput")
    wkt = nc.dram_tensor("wkt", [D, HG], BF16, kind="ExternalInput")
    wvt = nc.dram_tensor("wvt", [D, HG], BF16, kind="ExternalInput")
    bqp = nc.dram_tensor("bqp", [HG], F32, kind="ExternalInput")
    bkr = nc.dram_tensor("bkr", [1, HG], BF16, kind="ExternalInput")
    bvr = nc.dram_tensor("bvr", [1, HG], BF16, kind="ExternalInput")
    outd = nc.dram_tensor("out", [T, HG], F32, kind="ExternalOutput")

    with tile.TileContext(nc) as tc:
        with (
            tc.tile_pool(name="const", bufs=1) as constp,
            tc.tile_pool(name="wp", bufs=1) as wp,
            tc.tile_pool(name="xTp", bufs=2) as xTp,
            tc.tile_pool(name="kfp", bufs=1) as kfp,
            tc.tile_pool(name="vap", bufs=1) as vap,
            tc.tile_pool(name="qfp", bufs=1) as qfp,
            tc.tile_pool(name="kvsp", bufs=1) as kvsp,
            tc.tile_pool(name="stage", bufs=3) as stage,
            tc.tile_pool(name="ptmp", bufs=2) as ptmp,
            tc.tile_pool(name="outp", bufs=2) as outp,
            tc.tile_pool(name="rdp", bufs=2) as rdp,
            tc.tile_pool(name="tpsp", bufs=2, space="PSUM") as tpsp,
            tc.tile_pool(name="projp", bufs=3, space="PSUM") as projp,
            tc.tile_pool(name="kvpp", bufs=1, space="PSUM") as kvpp,
            tc.tile_pool(name="nmp", bufs=2, space="PSUM") as nmp,
        ):
            # first x tile DMA before anything else (startup latency)
            xs0 = stage.tile([128, D], F32, tag="xs")
            nc.sync.dma_start(xs0[:], xb.ap()[0:128, :])

            # ---- constants ----
            ident = constp.tile([128, 128], BF16)
            make_identity(nc, ident[:])
            mask_sb = constp.tile([128, TC], F32)
            nc.sync.dma_start(mask_sb[:], maskb.ap().rearrange("(a p) -> p a", p=128))
            bq_sb = constp.tile([128, OC], F32)
            nc.sync.dma_start(bq_sb[:], bqp.ap().rearrange("(a p) -> p a", p=128))
            if has_bias:
                ones_b = constp.tile([1, 128], BF16)
                nc.vector.memset(ones_b[:], 1.0)
                bk_sb = constp.tile([1, HG], BF16)
                nc.sync.dma_start(bk_sb[:], bkr.ap())
                bv_sb = constp.tile([1, HG], BF16)
                nc.sync.dma_start(bv_sb[:], bvr.ap())

            # ---- weights (host pre-transposed to [D, HG]) ----
            w_sb = {}
            for name, dram in (("q", wqt), ("k", wkt), ("v", wvt)):
                w = wp.tile([128, KC, HG], BF16, tag=f"w{name}")
                nc.sync.dma_start(w[:], dram.ap().rearrange("(kc p) n -> p kc n", p=128))
                w_sb[name] = w

            # ---- big persistent activations ----
            kf = kfp.tile([128, TC, HG], BF16)
            va = vap.tile([128, TC, OC * 130], BF16)
            qf = qfp.tile([128, OC, T], BF16)
            # kvs[:, j, :] = [kv_h0 (rows 0-63) | kv_h1 (rows 64-127) | z0 | z1],
            # complementary rows zero
            kvs = kvsp.tile([128, OC, 130], BF16)
            nc.vector.memset(kvs[:], 0.0)

            for half in range(HALVES):
                xT = xTp.tile([128, KC, T // HALVES], BF16, tag="xT")

                # ---- phase T: load + cast + transpose ----
                for tl in range(TCH if "T" in stages else 0):
                    t_c = half * TCH + tl
                    if t_c == 0:
                        xs = xs0
                    else:
                        xs = stage.tile([128, D], F32, tag="xs")
                        nc.sync.dma_start(
                            xs[:], xb.ap()[t_c * 128 : (t_c + 1) * 128, :]
                        )
                    xc = stage.tile([128, D], BF16, tag="xc")
                    nc.vector.tensor_copy(xc[:], xs[:])
                    if tmode == "dma":
                        for kc in range(KC):
                            nc.sync.dma_start_transpose(
                                xT[:, kc, tl * 128 : (tl + 1) * 128],
                                xc[:, kc * 128 : (kc + 1) * 128],
                            )
                    else:
                        tp = tpsp.tile([128, KC, 128], BF16, tag="tps")
                        for kc in range(KC):
                            nc.tensor.matmul(
                                tp[:, kc, :],
                                xc[:, kc * 128 : (kc + 1) * 128],
                                ident[:],
                                is_transpose=True,
                                start=(kc == 0),
                                stop=(kc == KC - 1),
                            )
                        dst = xT[:, :, tl * 128 : (tl + 1) * 128]
                        if tl % 2 == 0:
                            nc.vector.tensor_copy(dst, tp[:])
                        else:
                            nc.scalar.copy(dst, tp[:])

                # ---- phase A: k/v projections (token-major) + phi/mask ----
                for tl in range(TCH if "A" in stages else 0):
                    t_c = half * TCH + tl
                    m_col = mask_sb[:, t_c : t_c + 1]

                    kp = projp.tile([128, 512], F32, tag="big")
                    for kc in range(KC):
                        nc.tensor.matmul(
                            kp[:],
                            xT[:, kc, tl * 128 : (tl + 1) * 128],
                            w_sb["k"][:, kc, :],
                            start=(kc == 0),
                            stop=(kc == KC - 1 and not has_bias),
                        )
                    if has_bias:
                        nc.tensor.matmul(
                            kp[:], ones_b[:], bk_sb[:], start=False, stop=True
                        )
                    kmin = ptmp.tile([128, 512], F32, tag="mn")
                    nc.vector.tensor_scalar_min(kmin[:], kp[:], 0.0)
                    ke = ptmp.tile([128, 512], F32, tag="ex")
                    nc.scalar.activation(ke[:], kmin[:], AF.Exp)
                    kr = ptmp.tile([128, 512], F32, tag="rl")
                    # relu(k * m) == m * relu(k) for m >= 0
                    nc.scalar.activation(kr[:], kp[:], AF.Relu, scale=m_col)
                    nc.vector.scalar_tensor_tensor(
                        kf[:, t_c, :], ke[:], m_col, kr[:], op0=ALU.mult, op1=ALU.add
                    )

                    vp = projp.tile([128, 512], F32, tag="big")
                    for kc in range(KC):
                        nc.tensor.matmul(
                            vp[:],
                            xT[:, kc, tl * 128 : (tl + 1) * 128],
                            w_sb["v"][:, kc, :],
                            start=(kc == 0),
                            stop=(kc == KC - 1 and not has_bias),
                        )
                    if has_bias:
                        nc.tensor.matmul(
                            vp[:], ones_b[:], bv_sb[:], start=False, stop=True
                        )
                    va_t = va[:, t_c, :].rearrange("p (j h c) -> p j h c", h=2, c=65)
                    nc.scalar.mul(
                        va_t[:, :, :, 0:64],
                        vp[:].rearrange("p (j h c) -> p j h c", h=2, c=64),
                        m_col,
                    )
                    nc.vector.tensor_copy(
                        va_t[:, :, :, 64:65], m_col.broadcast_to((128, OC, 2, 1))
                    )

                # ---- phase C: q projection (head-major) + phi ----
                for oc in range(OC if "C" in stages else 0):
                    for t5l in range(T5H):
                        t5 = half * T5H + t5l
                        qp = projp.tile([128, 512], F32, tag="big")
                        for kc in range(KC):
                            nc.tensor.matmul(
                                qp[:],
                                w_sb["q"][:, kc, oc * 128 : (oc + 1) * 128],
                                xT[:, kc, t5l * 512 : (t5l + 1) * 512],
                                start=(kc == 0),
                                stop=(kc == KC - 1),
                            )
                        b_col = bq_sb[:, oc : oc + 1]
                        qmin = ptmp.tile([128, 512], F32, tag="mn")
                        if has_bias:
                            nc.vector.tensor_scalar(
                                qmin[:], qp[:], b_col, 0.0, op0=ALU.add, op1=ALU.min
                            )
                        else:
                            nc.vector.tensor_scalar_min(qmin[:], qp[:], 0.0)
                        qe = ptmp.tile([128, 512], F32, tag="ex")
                        nc.scalar.activation(qe[:], qmin[:], AF.Exp)
                        qr = ptmp.tile([128, 512], F32, tag="rl")
                        if has_bias:
                            nc.scalar.activation(qr[:], qp[:], AF.Relu, bias=b_col)
                        else:
                            nc.scalar.activation(qr[:], qp[:], AF.Relu)
                        nc.vector.tensor_add(
                            qf[:, oc, t5 * 512 : (t5 + 1) * 512], qe[:], qr[:]
                        )

            # ---- phase B: per-pair KV accumulation over all t ----
            for j in range(OC if "B" in stages else 0):
                kvp_t = kvpp.tile([128, 130], F32, tag="kvp")
                for t_c in range(TC):
                    nc.tensor.matmul(
                        kvp_t[:],
                        kf[:, t_c, j * 128 : (j + 1) * 128],
                        va[:, t_c, j * 130 : (j + 1) * 130],
                        start=(t_c == 0),
                        stop=(t_c == TC - 1),
                    )
                # rows 0-63: head 2j from cols 0-64; rows 64-127: head 2j+1
                # from cols 65-129 (col 64 in rows 64-127 equals z_h1 too)
                nc.vector.tensor_copy(kvs[0:64, j, 0:64], kvp_t[0:64, 0:64])
                nc.vector.tensor_copy(kvs[0:64, j, 128:129], kvp_t[0:64, 64:65])
                nc.vector.tensor_copy(kvs[64:128, j, 64:128], kvp_t[64:128, 65:129])
                nc.vector.tensor_copy(kvs[64:128, j, 129:130], kvp_t[64:128, 64:65])

            # ---- phase D: fused num+den + normalize + store ----
            for t_c in range(TC if "D" in stages else 0):
                m_col = mask_sb[:, t_c : t_c + 1]
                nms = []
                for jj in range(2):  # two pairs per PSUM bank tile
                    nm2 = nmp.tile([128, 2, 130], F32, tag="nm")
                    for j2 in range(2):
                        nc.tensor.matmul(
                            nm2[:, j2, :],
                            qf[:, jj * 2 + j2, t_c * 128 : (t_c + 1) * 128],
                            kvs[:, jj * 2 + j2, :],
                            start=True,
                            stop=True,
                        )
                    nms.append(nm2)
                den8 = rdp.tile([128, 8], F32, tag="den8")
                for jj in range(2):
                    nc.vector.tensor_copy(
                        den8[:, jj * 4 : (jj + 1) * 4],
                        nms[jj][:, :, 128:130],
                    )
                rden = rdp.tile([128, 8], F32, tag="rd")
                nc.vector.tensor_scalar_max(rden[:], den8[:], 1e-6)
                nc.vector.reciprocal(rden[:], rden[:])
                nc.vector.tensor_scalar_mul(rden[:], rden[:], m_col)

                ot = outp.tile([128, HG], F32, tag="ot")
                for jj in range(2):
                    for j2 in range(2):
                        for hh in range(2):
                            h = (jj * 2 + j2) * 2 + hh
                            nc.vector.tensor_scalar_mul(
                                ot[:, h * 64 : (h + 1) * 64],
                                nms[jj][:, j2, hh * 64 : (hh + 1) * 64],
                                rden[:, h : h + 1],
                            )
                nc.sync.dma_start(outd.ap()[t_c * 128 : (t_c + 1) * 128, :], ot[:])

    nc.compile()
    return nc


_PROGRAM_CACHE = {}


def _get_program(has_bias: bool):
    if has_bias not in _PROGRAM_CACHE:
        _PROGRAM_CACHE[has_bias] = _build_program(has_bias)
    return _PROGRAM_CACHE[has_bias]


def _prep_inputs(x, mask, Wq, bq, Wk, bk, Wv, bv):
    """Slice + lay out per-core inputs. Core c -> batch c//2, head-group c%2."""
    bf16 = ml_dtypes.bfloat16
    in_maps = []
    for c in range(8):
        bi, hg = c // 2, c % 2
        sl = slice(hg * HG, (hg + 1) * HG)
        in_maps.append(
            {
                "xb": np.ascontiguousarray(x[bi]).astype(np.float32, copy=False),
                "maskb": np.ascontiguousarray(mask[bi]).astype(np.float32, copy=False),
                "wqt": np.ascontiguousarray(Wq[sl, :].T).astype(bf16),
                "wkt": np.ascontiguousarray(Wk[sl, :].T).astype(bf16),
                "wvt": np.ascontiguousarray(Wv[sl, :].T).astype(bf16),
                "bqp": np.ascontiguousarray(bq[sl]).astype(np.float32, copy=False),
                "bkr": np.ascontiguousarray(bk[sl]).astype(bf16).reshape(1, HG),
                "bvr": np.ascontiguousarray(bv[sl]).astype(bf16).reshape(1, HG),
            }
        )
    return in_maps


def kernel(x, mask, Wq, bq, Wk, bk, Wv, bv, n_heads, **run_kwargs):
    x = np.asarray(x)
    mask = np.asarray(mask)
    Wq, bq = np.asarray(Wq), np.asarray(bq)
    Wk, bk = np.asarray(Wk), np.asarray(bk)
    Wv, bv = np.asarray(Wv), np.asarray(bv)
    b, t, d = x.shape
    assert (b, t, d) == (4, T, D) and int(n_heads) == 16, (
        f"kernel hardcoded for (4,{T},{D}) h=16, got {(b, t, d)} h={n_heads}"
    )

    has_bias = bool(np.any(bq) or np.any(bk) or np.any(bv))
    nc = _get_program(has_bias)
    in_maps = _prep_inputs(x, mask, Wq, bq, Wk, bk, Wv, bv)
    res = run_bass_kernel_spmd(nc, in_maps, core_ids=list(range(8)), **run_kwargs)

    out = np.empty((4, T, D), dtype=np.float32)
    for c in range(8):
        bi, hg = c // 2, c % 2
        out[bi, :, hg * HG : (hg + 1) * HG] = res.results[c]["out"]
    if run_kwargs:
        kernel.last_results = res
    return out
